# revision 49
# baseline (speedup 1.0000x reference)
"""Multi-head attention (B=4, S=2048, D=768, H=12) on 8 TRN2 NeuronCores.

Sharding: core = (batch b, query-half). Host permutes each core's sequence so
its 1024 query rows come FIRST (softmax over keys is permutation-invariant;
RoPE uses the permuted position ids, so this is exact). Each core computes
full-sequence K/V for its batch plus Q for rows 0:1024, then SDPA + o_proj.
Zero collectives; output rows disjoint across cores.

Structure (single fused phase, engines balanced):
 - hs loaded DIRECTLY in T-layout via DMA xbar transpose (no PE transposes)
 - rope tables built on device from position ids (rank-1 matmul + mod-1 via
   int cast + Sin LUT); Q tables = prefix columns of the K tables
 - K/Q projection per e-chunk in T-layout, bias added via rank-1 PE matmul,
   psum evicted by DVE; rope applied in T-layout (partition-shifted copy)
 - SDPA head-pair-major with q-half sub-loops: scores psum [128,1024]
   (head0|head1 in adjacent banks), ONE exp ACT per (skt, qhalf) covering
   both heads; PV accumulates [65,512] per head with ones-column rowsum
 - e-chunk hp+1 projections + V projections drained into PE gaps during
   SDPA at ~1 closure per skt step (ScalarE exp stream stays dense)
 - normalization: rowsum -> reciprocal on 128 lanes -> partition_broadcast
   -> DVE scale into attnT; o_proj row-major at the tail
"""

from collections import deque
from contextlib import ExitStack

import numpy as np

import concourse.bass as bass
import concourse.bacc as bacc
import concourse.mybir as mybir
import concourse.tile as tile
from concourse.bass import ds, ts
from concourse.bass_utils import run_bass_kernel_spmd

F32 = mybir.dt.float32
BF16 = mybir.dt.bfloat16
I32 = mybir.dt.int32
AF = mybir.ActivationFunctionType

B, S, D, H = 4, 2048, 768, 12
HD = 64
SQ = 1024          # query rows per core (prefix of permuted seq)
DC = D // 128      # 6 d-chunks
ST = S // 128      # 16 key tiles of 128
HP = 6             # head pairs
VW = 784           # Vaug width: 12*65=780 used + pad to 16-multiple
ROPE_BASE = 10000.0
TWO_PI = float(2.0 * np.pi)
N_CORES = 8
LAG = 6


DEBUG = False


def build_nc():
    nc = bacc.Bacc("TRN2", target_bir_lowering=False, debug=False,
                   num_devices=N_CORES)

    hs = nc.dram_tensor("hs", [S, D], BF16, kind="ExternalInput")
    pos = nc.dram_tensor("pos", [1, S], I32, kind="ExternalInput")
    wqT = nc.dram_tensor("wqT", [D, D], BF16, kind="ExternalInput")
    wkT = nc.dram_tensor("wkT", [D, D], BF16, kind="ExternalInput")
    wvT = nc.dram_tensor("wvT", [D, D], BF16, kind="ExternalInput")
    woT = nc.dram_tensor("woT", [D, D], BF16, kind="ExternalInput")
    bq = nc.dram_tensor("bq", [1, D], F32, kind="ExternalInput")
    bk = nc.dram_tensor("bk", [1, D], F32, kind="ExternalInput")
    bv = nc.dram_tensor("bv", [1, D], F32, kind="ExternalInput")
    out = nc.dram_tensor("out", [SQ, D], F32, kind="ExternalOutput")

    invf_turns_np = ((1.0 / ROPE_BASE) ** (np.arange(32) / 32.0) / TWO_PI
                     ).astype(np.float32)
    invf_dram = nc.inline_tensor(
        np.tile(invf_turns_np, 4).reshape(1, 128), name="invf_turns")

    dbg = None
    if DEBUG:
        dbg = {
            "xt_d": nc.dram_tensor("xt_d", [D, S], BF16,
                                   kind="ExternalOutput"),
            "kt_d": nc.dram_tensor("kt_d", [D, S], BF16,
                                   kind="ExternalOutput"),
            "qt_d": nc.dram_tensor("qt_d", [D, SQ], BF16,
                                   kind="ExternalOutput"),
            "va_d": nc.dram_tensor("va_d", [ST * 128, VW], BF16,
                                   kind="ExternalOutput"),
            "ex_d": nc.dram_tensor("ex_d", [ST * 128, 1024], BF16,
                                   kind="ExternalOutput"),
            "at_d": nc.dram_tensor("at_d", [D, SQ], BF16,
                                   kind="ExternalOutput"),
            # evacuated pv (numerator + Z row) and broadcast 1/Z for hp0
            "pv_d": nc.dram_tensor("pv_d", [2 * 2 * 65, 512], F32,
                                   kind="ExternalOutput"),
            "rb_d": nc.dram_tensor("rb_d", [2 * 2 * 64, 512], F32,
                                   kind="ExternalOutput"),
        }

    with tile.TileContext(nc) as tc:
        _body(nc, tc, hs, pos, wqT, wkT, wvT, woT, bq, bk, bv, out,
              invf_dram, dbg)
    nc.compile()
    return nc


def _body(nc, tc, hs, pos, wqT, wkT, wvT, woT, bq, bk, bv, out, invf_dram,
          dbg=None):
  with ExitStack() as ctx:
    # ---- long-lived pools, opened first so they own the low addresses ----
    const = ctx.enter_context(tc.tile_pool(name="const", bufs=1))
    persist = ctx.enter_context(tc.tile_pool(name="persist", bufs=1))
    xt_pool = ctx.enter_context(tc.tile_pool(name="xt", bufs=1))
    wpool = ctx.enter_context(tc.tile_pool(name="w", bufs=1))
    expool = ctx.enter_context(tc.tile_pool(name="ex", bufs=16))
    nrm = ctx.enter_context(tc.tile_pool(name="nrm", bufs=1))
    osb = ctx.enter_context(tc.tile_pool(name="osb", bufs=2))
    shp_pool = ctx.enter_context(tc.tile_pool(name="shift", bufs=1))
    scps = ctx.enter_context(tc.tile_pool(name="scps", bufs=2, space="PSUM"))
    pvps = ctx.enter_context(tc.tile_pool(name="pvps", bufs=1, space="PSUM"))
    prps = ctx.enter_context(tc.tile_pool(name="prps", bufs=2, space="PSUM"))

    # ---- constants / input DMAs ----
    ones_row = const.tile([1, 512], BF16, tag="ones_row")
    nc.gpsimd.memset(ones_row[:], 1.0)
    invf = const.tile([1, 128], F32, tag="invf")
    nc.sync.dma_start(invf[:], invf_dram[:])
    posi = const.tile([1, S], I32, tag="posi")
    nc.sync.dma_start(posi[:], pos[:])

    # biases first on the gpsimd ring (SWDGE cast-on-DMA f32 -> bf16)
    bq_b = const.tile([1, D], BF16, tag="bq_b")
    bk_b = const.tile([1, D], BF16, tag="bk_b")
    bv_b = const.tile([1, D], BF16, tag="bv_b")
    nc.gpsimd.dma_start(bq_b[:], bq[:])
    nc.gpsimd.dma_start(bk_b[:], bk[:])
    nc.gpsimd.dma_start(bv_b[:], bv[:])

    # whole-weight batched loads on the SWDGE queue (one DMA each);
    # chunk dc lives at cols [dc*768, (dc+1)*768)
    def load_weight(wT_dram, name, pool):
        w_all = pool.tile([128, DC * D], BF16, tag=f"w_{name}",
                          name=f"w_{name}")
        nc.gpsimd.dma_start(
            w_all[:].rearrange("p (c j) -> p c j", c=DC),
            wT_dram[:].rearrange("(c p) j -> p c j", p=128))
        return w_all

    wk_sb = load_weight(wkT, "k", wpool)
    wv_sb = load_weight(wvT, "v", wpool)
    wq_sb = load_weight(wqT, "q", wpool)

    # ---- hidden states: DMA xbar transpose straight into T-layout ----
    xT = [xt_pool.tile([128, S], BF16, tag=f"xT{dc}", name=f"xT{dc}")
          for dc in range(DC)]
    for sg in range(4):
        for dc in range(DC):
            nc.sync.dma_start(xT[dc][:, ts(sg, 512)],
                              hs[ts(sg, 512), ts(dc, 128)], transpose=True)

    # ---- persistent activation tensors ----
    QT = [persist.tile([128, SQ], BF16, tag=f"QT{e}", name=f"QT{e}")
          for e in range(DC)]
    KT = [persist.tile([128, S], BF16, tag=f"KT{e}", name=f"KT{e}")
          for e in range(DC)]
    Vaug = [persist.tile([128, VW], BF16, tag=f"Vaug{st}",
                         name=f"Vaug{st}") for st in range(ST)]
    attnT = [persist.tile([128, SQ], BF16, tag=f"attnT{e}",
                          name=f"attnT{e}") for e in range(DC)]
    cosR = persist.tile([128, S], BF16, tag="cosR")
    sinS = persist.tile([128, S], BF16, tag="sinS")

    # ---- rope tables from position ids (k tables; q rows are the prefix) --
    # built at full 128-partition width via the 4x-replicated invf row, so
    # the Sin ACT writes cosR/sinS slices directly (no cross-partition copy)
    with tc.tile_pool(name="rope_sb", bufs=1) as rp:
        for ch in range(4):           # 512-col chunks
            posf = rp.tile([1, 512], F32, tag="posf")
            nc.vector.tensor_copy(posf[:], posi[:, ts(ch, 512)])
            turns = scps.tile([128, 512], F32, tag="sc", name="turns")
            nc.tensor.matmul(turns[:], invf[:], posf[:], start=True,
                             stop=True)
            for dst, shift in ((sinS, 0.0), (cosR, 0.25)):
                tsh = rp.tile([128, 512], F32, tag="tsh")
                nc.vector.tensor_scalar_add(tsh[:], turns[:], shift)
                ti = rp.tile([128, 512], I32, tag="ti")
                nc.vector.tensor_copy(ti[:], tsh[:])
                tif = rp.tile([128, 512], F32, tag="tif")
                nc.vector.tensor_copy(tif[:], ti[:])
                nc.vector.tensor_sub(tsh[:], tsh[:], tif[:])
                nc.scalar.activation(dst[:, ts(ch, 512)], tsh[:], AF.Sin,
                                     scale=TWO_PI)
            # rotate-half sign: rows 0-31 and 64-95 carry -sin
            for q in (0, 2):
                nc.vector.tensor_scalar_mul(
                    sinS[ds(32 * q, 32), ts(ch, 512)],
                    sinS[ds(32 * q, 32), ts(ch, 512)], -1.0)
            if ch == 0:
                # preload the exp table set off the critical path
                dummy = rp.tile([32, 32], BF16, tag="dummy_exp")
                nc.scalar.activation(dummy[:], sinS[ds(0, 32), ds(0, 32)],
                                     AF.Exp)

        # ---- projection helpers ----
        def kq_slice(dst, w_sb, b_b, e, i):
            # one 512-col T-layout projection slice for e-chunk e
            p = prps.tile([128, 512], F32, tag="proj", name="proj_p")
            for dc in range(DC):
                nc.tensor.matmul(p[:], w_sb[:, ds(dc * D + e * 128, 128)],
                                 xT[dc][:, ts(i, 512)],
                                 start=(dc == 0), stop=False)
            nc.tensor.matmul(p[:], b_b[:, ts(e, 128)], ones_row[:],
                             start=False, stop=True)
            nc.vector.tensor_copy(dst[e][:, ts(i, 512)], p[:])

        def rope_slice(dst, e, lo, n, eng=None):
            # in-place rope on dst[e][:, lo:lo+n] using cosR/sinS cols
            sh = shp_pool.tile([128, 1024], BF16, tag="shift", name="sh")
            for q in range(4):
                src_q = (q // 2) * 2 + (1 - q % 2)  # 0<->32, 64<->96
                (eng or nc.sync).dma_start(sh[ds(32 * q, 32), 0:n],
                                           dst[e][ds(32 * src_q, 32),
                                                  ds(lo, n)])
            tmp = shp_pool.tile([128, 1024], BF16, tag="ropetmp",
                                name="ropetmp")
            nc.vector.tensor_mul(tmp[:, 0:n], sh[:, 0:n],
                                 sinS[:, ds(lo, n)])
            nc.vector.tensor_mul(dst[e][:, ds(lo, n)], dst[e][:, ds(lo, n)],
                                 cosR[:, ds(lo, n)])
            nc.vector.tensor_add(dst[e][:, ds(lo, n)], dst[e][:, ds(lo, n)],
                                 tmp[:, 0:n])

        def v_proj_nt(st, nt):
            p = prps.tile([128, 512], F32, tag="proj", name="vproj_p")
            for dc in range(DC):
                nc.tensor.matmul(p[:, 0:384], xT[dc][:, ts(st, 128)],
                                 wv_sb[:, ds(dc * D + nt * 384, 384)],
                                 start=(dc == 0), stop=False)
            nc.tensor.matmul(p[:, 0:384], ones_row[:, 0:128],
                             bv_b[:, ts(nt, 384)], start=False, stop=True)
            dst = Vaug[st][:, 0:H * 65].rearrange("p (h x) -> p h x", x=65)
            nc.vector.tensor_copy(
                dst[:, ds(nt * 6, 6), 0:64],
                p[:, 0:384].rearrange("p (h hd) -> p h hd", hd=64))

        def v_finish(st):
            va = Vaug[st][:, 0:H * 65].rearrange("p (h x) -> p h x", x=65)
            nc.gpsimd.memset(va[:, :, 64:65], 1.0)
            nc.gpsimd.memset(Vaug[st][:, H * 65:], 0.0)

        # ---- prologue: e=0 Q then K (pipelined w/ transposes) then V;
        # rope-shift DMAs ride the idle scalar HWDGE ring ----
        for i in range(2):
            kq_slice(QT, wq_sb, bq_b, 0, i)
            rope_slice(QT, 0, i * 512, 512, eng=nc.scalar)
        for sg in range(4):
            kq_slice(KT, wk_sb, bk_b, 0, sg)
            rope_slice(KT, 0, sg * 512, 512, eng=nc.scalar)
        for st in range(4):
            for nt in range(2):
                v_proj_nt(st, nt)
            v_finish(st)

    def v_all():
        for st in range(4, ST):
            for nt in range(2):
                v_proj_nt(st, nt)
            v_finish(st)

    # wo loads late; its pool reuses the (closed) rope scratch range
    wop = ctx.enter_context(tc.tile_pool(name="wop", bufs=1))
    wo_sb = load_weight(woT, "o", wop)

    # ---- deferred work closures (drained during SDPA) ----
    def make_work(hp):
        items = deque()
        if hp < HP - 1:
            e = hp + 1
            for sg in range(4):
                items.append(lambda e=e, sg=sg: kq_slice(KT, wk_sb, bk_b,
                                                         e, sg))
                items.append(lambda e=e, sg=sg: rope_slice(KT, e, sg * 512,
                                                           512))
            for i in range(2):
                items.append(lambda e=e, i=i: kq_slice(QT, wq_sb, bq_b,
                                                       e, i))
                items.append(lambda e=e, i=i: rope_slice(QT, e, i * 512,
                                                         512))
        return items

    def o_tile(st, nt):
        p = prps.tile([128, 512], F32, tag="proj", name="o_p")
        for dc in range(DC):
            nc.tensor.matmul(p[:, 0:384], attnT[dc][:, ts(st, 128)],
                             wo_sb[:, ds(dc * D + nt * 384, 384)],
                             start=(dc == 0), stop=(dc == DC - 1))
        o = osb.tile([128, 384], F32, tag="o_out", name="o_out")
        nc.vector.tensor_copy(o[:], p[:, 0:384])
        nc.sync.dma_start(out[ts(st, 128), ts(nt, 384)], o[:])

    # ---- SDPA ----
    pending_norm = [None]
    for hp in range(HP):
        work = make_work(hp)
        steps_total = 32
        step_no = 0
        for qh in range(2):
            pv = [pvps.tile([128, 512], F32, tag=f"pv{i}", name=f"pv{i}")
                  for i in range(2)]
            exs = [None] * ST

            def do_pv(k, pv=pv, exs=exs, hp=hp):
                for i in range(2):
                    h = 2 * hp + i
                    nc.tensor.matmul(pv[i][ds(0, 65), :],
                                     Vaug[k][:, ds(h * 65, 65)],
                                     exs[k][:, ds(i * 512, 512)],
                                     start=(k == 0), stop=(k == ST - 1))

            for skt in range(ST):
                sc = scps.tile([128, 1024], F32, tag="sc", name="sc")
                for i in range(2):
                    nc.tensor.matmul(
                        sc[:, ds(i * 512, 512)],
                        KT[hp][ds(64 * i, 64), ts(skt, 128)],
                        QT[hp][ds(64 * i, 64), ds(qh * 512, 512)],
                        start=True, stop=True, tile_position=(64 * i, 0))
                e = expool.tile([128, 1024], BF16, tag="ex", name="expt")
                nc.scalar.activation(e[:], sc[:], AF.Exp, scale=0.125)
                exs[skt] = e
                if dbg is not None and hp == 0 and qh == 0:
                    nc.sync.dma_start(dbg["ex_d"][ts(skt, 128), :], e[:])
                if skt == 2 and pending_norm[0] is not None:
                    # previous sub-iteration's normalize, emitted here so
                    # its psum reads execute with slack
                    pending_norm[0]()
                    pending_norm[0] = None
                if not (hp == 0 and qh == 0) and skt >= LAG:
                    do_pv(skt - LAG)
                # drain deferred projection work into the PE gap
                step_no += 1
                n_drain = min(len(work), 1)
                for _ in range(n_drain):
                    work.popleft()()
            if hp == 0 and qh == 0:
                # V projections emitted AFTER the qh0 score/exp stream (so
                # scores outrank them on the PE) and BEFORE its PV, which
                # then executes promptly once Vaug lands
                v_all()
                for k in range(ST):
                    do_pv(k)
            else:
                for k in range(ST - LAG, ST):
                    do_pv(k)
            if hp == HP - 1 and qh == 1:
                # o_proj for the qh=0 output rows can start as soon as the
                # last qh=0 norm (emitted at skt==2 above) lands
                for st_ in range(4):
                    for nt_ in range(2):
                        o_tile(st_, nt_)

            # normalize (deferred): evacuate psum, rowsum recip, scale
            def norm_fn(pv=pv, hp=hp, qh=qh):
                for i in range(2):
                    rsrow = nrm.tile([1, 512], F32, tag="rsrow",
                                     name="rsrow")
                    nc.vector.tensor_copy(rsrow[:], pv[i][ds(64, 1), :])
                    pvs = nrm.tile([64, 512], F32, tag="pvs", name="pvs")
                    nc.vector.tensor_copy(pvs[:], pv[i][ds(0, 64), :])
                    c4 = nrm.tile([128, 4], F32, tag="c4", name="c4")
                    nc.sync.dma_start(c4[:], rsrow[:])
                    r4 = nrm.tile([128, 4], F32, tag="r4", name="r4")
                    nc.vector.reciprocal(r4[:], c4[:])
                    recb = nrm.tile([1, 512], F32, tag="recb", name="recb")
                    nc.sync.dma_start(recb[:], r4[:])
                    rbs = nrm.tile([64, 512], F32, tag="rbs", name="rbs")
                    nc.gpsimd.partition_broadcast(rbs[:], recb[:])
                    nc.vector.tensor_mul(
                        attnT[hp][ds(64 * i, 64), ds(qh * 512, 512)],
                        pvs[:], rbs[:])
                    if dbg is not None and hp == 0:
                        blk = (qh * 2 + i)
                        nc.sync.dma_start(
                            dbg["pv_d"][ds(blk * 65, 64), :], pvs[:])
                        nc.sync.dma_start(
                            dbg["pv_d"][ds(blk * 65 + 64, 1), :], rsrow[:])
                        nc.sync.dma_start(
                            dbg["rb_d"][ds(blk * 64, 64), :], rbs[:])

            pending_norm[0] = norm_fn
    pending_norm[0]()

    # ---- o_proj remainder (qh=1 output rows) ----
    for st in range(4, SQ // 128):
        for nt in range(2):
            o_tile(st, nt)

    if dbg is not None:
        for dc in range(DC):
            nc.sync.dma_start(dbg["xt_d"][ts(dc, 128), :], xT[dc][:])
            nc.sync.dma_start(dbg["kt_d"][ts(dc, 128), :], KT[dc][:])
            nc.sync.dma_start(dbg["qt_d"][ts(dc, 128), :], QT[dc][:])
            nc.sync.dma_start(dbg["at_d"][ts(dc, 128), :], attnT[dc][:])
        for st in range(ST):
            nc.sync.dma_start(dbg["va_d"][ts(st, 128), :], Vaug[st][:])


_NC_CACHE = None


def _get_nc():
    global _NC_CACHE
    if _NC_CACHE is None:
        _NC_CACHE = build_nc()
    return _NC_CACHE


def kernel(hidden_states, position_ids, wq, bq, wk, bk, wv, bv, wo,
           _trace=False):
    import ml_dtypes
    bf16 = ml_dtypes.bfloat16
    hidden_states = np.asarray(hidden_states, dtype=np.float32).astype(bf16)
    position_ids = np.asarray(position_ids, dtype=np.int32)
    wqT = np.ascontiguousarray(np.asarray(wq, np.float32).T.astype(bf16))
    wkT = np.ascontiguousarray(np.asarray(wk, np.float32).T.astype(bf16))
    wvT = np.ascontiguousarray(np.asarray(wv, np.float32).T.astype(bf16))
    woT = np.ascontiguousarray(np.asarray(wo, np.float32).T.astype(bf16))
    bq_r = np.ascontiguousarray(np.asarray(bq, np.float32).reshape(1, D))
    bk_r = np.ascontiguousarray(np.asarray(bk, np.float32).reshape(1, D))
    bv_r = np.ascontiguousarray(np.asarray(bv, np.float32).reshape(1, D))

    nc = _get_nc()
    in_maps = []
    for core in range(N_CORES):
        b, half = core // 2, core % 2
        if half == 0:
            hsp = hidden_states[b]
            posp = position_ids[b]
        else:
            hsp = np.concatenate([hidden_states[b, SQ:],
                                  hidden_states[b, :SQ]], axis=0)
            posp = np.concatenate([position_ids[b, SQ:],
                                   position_ids[b, :SQ]])
        in_maps.append({
            "hs": np.ascontiguousarray(hsp),
            "pos": np.ascontiguousarray(posp.reshape(1, S)),
            "wqT": wqT, "wkT": wkT, "wvT": wvT, "woT": woT,
            "bq": bq_r, "bk": bk_r, "bv": bv_r,
        })
    res = run_bass_kernel_spmd(nc, in_maps, list(range(N_CORES)),
                               trace=_trace)
    if DEBUG:
        kernel._last_dbg_results = res
    outp = np.empty((B, S, D), np.float32)
    for core in range(N_CORES):
        b, half = core // 2, core % 2
        outp[b, half * SQ:(half + 1) * SQ] = res.results[core]["out"]
    if _trace:
        kernel._last_exec_time_ns = res.exec_time_ns
        kernel._last_results = res
    return outp


# revision 50
# speedup vs baseline: 1.0023x; 1.0023x over previous
"""Multi-head attention (B=4, S=2048, D=768, H=12) on 8 TRN2 NeuronCores.

Sharding: core = (batch b, query-half). Host permutes each core's sequence so
its 1024 query rows come FIRST (softmax over keys is permutation-invariant;
RoPE uses the permuted position ids, so this is exact). Each core computes
full-sequence K/V for its batch plus Q for rows 0:1024, then SDPA + o_proj.
Zero collectives; output rows disjoint across cores.

Structure (single fused phase, engines balanced):
 - hs loaded DIRECTLY in T-layout via DMA xbar transpose (no PE transposes)
 - rope tables built on device from position ids (rank-1 matmul + mod-1 via
   int cast + Sin LUT); Q tables = prefix columns of the K tables
 - K/Q projection per e-chunk in T-layout, bias added via rank-1 PE matmul,
   psum evicted by DVE; rope applied in T-layout (partition-shifted copy)
 - SDPA head-pair-major with q-half sub-loops: scores psum [128,1024]
   (head0|head1 in adjacent banks), ONE exp ACT per (skt, qhalf) covering
   both heads; PV accumulates [65,512] per head with ones-column rowsum
 - e-chunk hp+1 projections + V projections drained into PE gaps during
   SDPA at ~1 closure per skt step (ScalarE exp stream stays dense)
 - normalization: rowsum -> reciprocal on 128 lanes -> partition_broadcast
   -> DVE scale into attnT; o_proj row-major at the tail
"""

from collections import deque
from contextlib import ExitStack

import numpy as np

import concourse.bass as bass
import concourse.bacc as bacc
import concourse.mybir as mybir
import concourse.tile as tile
from concourse.bass import ds, ts
from concourse.bass_utils import run_bass_kernel_spmd

F32 = mybir.dt.float32
BF16 = mybir.dt.bfloat16
I32 = mybir.dt.int32
AF = mybir.ActivationFunctionType

B, S, D, H = 4, 2048, 768, 12
HD = 64
SQ = 1024          # query rows per core (prefix of permuted seq)
DC = D // 128      # 6 d-chunks
ST = S // 128      # 16 key tiles of 128
HP = 6             # head pairs
VW = 784           # Vaug width: 12*65=780 used + pad to 16-multiple
ROPE_BASE = 10000.0
TWO_PI = float(2.0 * np.pi)
N_CORES = 8
LAG = 6


DEBUG = False


def build_nc():
    nc = bacc.Bacc("TRN2", target_bir_lowering=False, debug=False,
                   num_devices=N_CORES)

    hs = nc.dram_tensor("hs", [S, D], BF16, kind="ExternalInput")
    pos = nc.dram_tensor("pos", [1, S], I32, kind="ExternalInput")
    wqT = nc.dram_tensor("wqT", [D, D], BF16, kind="ExternalInput")
    wkT = nc.dram_tensor("wkT", [D, D], BF16, kind="ExternalInput")
    wvT = nc.dram_tensor("wvT", [D, D], BF16, kind="ExternalInput")
    woT = nc.dram_tensor("woT", [D, D], BF16, kind="ExternalInput")
    bq = nc.dram_tensor("bq", [1, D], F32, kind="ExternalInput")
    bk = nc.dram_tensor("bk", [1, D], F32, kind="ExternalInput")
    bv = nc.dram_tensor("bv", [1, D], F32, kind="ExternalInput")
    out = nc.dram_tensor("out", [SQ, D], F32, kind="ExternalOutput")

    invf_turns_np = ((1.0 / ROPE_BASE) ** (np.arange(32) / 32.0) / TWO_PI
                     ).astype(np.float32)
    invf_dram = nc.inline_tensor(
        np.tile(invf_turns_np, 4).reshape(1, 128), name="invf_turns")

    dbg = None
    if DEBUG:
        dbg = {
            "xt_d": nc.dram_tensor("xt_d", [D, S], BF16,
                                   kind="ExternalOutput"),
            "kt_d": nc.dram_tensor("kt_d", [D, S], BF16,
                                   kind="ExternalOutput"),
            "qt_d": nc.dram_tensor("qt_d", [D, SQ], BF16,
                                   kind="ExternalOutput"),
            "va_d": nc.dram_tensor("va_d", [ST * 128, VW], BF16,
                                   kind="ExternalOutput"),
            "ex_d": nc.dram_tensor("ex_d", [ST * 128, 1024], BF16,
                                   kind="ExternalOutput"),
            "at_d": nc.dram_tensor("at_d", [D, SQ], BF16,
                                   kind="ExternalOutput"),
            # evacuated pv (numerator + Z row) and broadcast 1/Z for hp0
            "pv_d": nc.dram_tensor("pv_d", [2 * 2 * 65, 512], F32,
                                   kind="ExternalOutput"),
            "rb_d": nc.dram_tensor("rb_d", [2 * 2 * 64, 512], F32,
                                   kind="ExternalOutput"),
        }

    with tile.TileContext(nc) as tc:
        _body(nc, tc, hs, pos, wqT, wkT, wvT, woT, bq, bk, bv, out,
              invf_dram, dbg)
    nc.compile()
    return nc


def _body(nc, tc, hs, pos, wqT, wkT, wvT, woT, bq, bk, bv, out, invf_dram,
          dbg=None):
  with ExitStack() as ctx:
    # ---- long-lived pools, opened first so they own the low addresses ----
    const = ctx.enter_context(tc.tile_pool(name="const", bufs=1))
    persist = ctx.enter_context(tc.tile_pool(name="persist", bufs=1))
    xt_pool = ctx.enter_context(tc.tile_pool(name="xt", bufs=1))
    wpool = ctx.enter_context(tc.tile_pool(name="w", bufs=1))
    expool = ctx.enter_context(tc.tile_pool(name="ex", bufs=16))
    nrm = ctx.enter_context(tc.tile_pool(name="nrm", bufs=1))
    osb = ctx.enter_context(tc.tile_pool(name="osb", bufs=2))
    shp_pool = ctx.enter_context(tc.tile_pool(name="shift", bufs=1))
    scps = ctx.enter_context(tc.tile_pool(name="scps", bufs=2, space="PSUM"))
    pvps = ctx.enter_context(tc.tile_pool(name="pvps", bufs=1, space="PSUM"))
    prps = ctx.enter_context(tc.tile_pool(name="prps", bufs=2, space="PSUM"))

    # ---- constants / input DMAs ----
    ones_row = const.tile([1, 512], BF16, tag="ones_row")
    nc.gpsimd.memset(ones_row[:], 1.0)
    invf = const.tile([1, 128], F32, tag="invf")
    nc.sync.dma_start(invf[:], invf_dram[:])
    posi = const.tile([1, S], I32, tag="posi")
    nc.sync.dma_start(posi[:], pos[:])

    # biases first on the gpsimd ring (SWDGE cast-on-DMA f32 -> bf16)
    bq_b = const.tile([1, D], BF16, tag="bq_b")
    bk_b = const.tile([1, D], BF16, tag="bk_b")
    bv_b = const.tile([1, D], BF16, tag="bv_b")
    nc.gpsimd.dma_start(bq_b[:], bq[:])
    nc.gpsimd.dma_start(bk_b[:], bk[:])
    nc.gpsimd.dma_start(bv_b[:], bv[:])

    # whole-weight batched loads on the SWDGE queue (one DMA each);
    # chunk dc lives at cols [dc*768, (dc+1)*768)
    def load_weight(wT_dram, name, pool):
        w_all = pool.tile([128, DC * D], BF16, tag=f"w_{name}",
                          name=f"w_{name}")
        nc.gpsimd.dma_start(
            w_all[:].rearrange("p (c j) -> p c j", c=DC),
            wT_dram[:].rearrange("(c p) j -> p c j", p=128))
        return w_all

    wk_sb = load_weight(wkT, "k", wpool)
    wv_sb = load_weight(wvT, "v", wpool)
    wq_sb = load_weight(wqT, "q", wpool)

    # ---- hidden states: DMA xbar transpose straight into T-layout ----
    xT = [xt_pool.tile([128, S], BF16, tag=f"xT{dc}", name=f"xT{dc}")
          for dc in range(DC)]
    for sg in range(4):
        for dc in range(DC):
            nc.sync.dma_start(xT[dc][:, ts(sg, 512)],
                              hs[ts(sg, 512), ts(dc, 128)], transpose=True)

    # ---- persistent activation tensors ----
    QT = [persist.tile([128, SQ], BF16, tag=f"QT{e}", name=f"QT{e}")
          for e in range(DC)]
    KT = [persist.tile([128, S], BF16, tag=f"KT{e}", name=f"KT{e}")
          for e in range(DC)]
    Vaug = [persist.tile([128, VW], BF16, tag=f"Vaug{st}",
                         name=f"Vaug{st}") for st in range(ST)]
    attnT = [persist.tile([128, SQ], BF16, tag=f"attnT{e}",
                          name=f"attnT{e}") for e in range(DC)]
    cosR = persist.tile([128, S], BF16, tag="cosR")
    sinS = persist.tile([128, S], BF16, tag="sinS")

    # ---- rope tables from position ids (k tables; q rows are the prefix) --
    # built at full 128-partition width via the 4x-replicated invf row, so
    # the Sin ACT writes cosR/sinS slices directly (no cross-partition copy)
    with tc.tile_pool(name="rope_sb", bufs=1) as rp:
        for ch in range(4):           # 512-col chunks
            posf = rp.tile([1, 512], F32, tag="posf")
            nc.vector.tensor_copy(posf[:], posi[:, ts(ch, 512)])
            turns = scps.tile([128, 512], F32, tag="sc", name="turns")
            nc.tensor.matmul(turns[:], invf[:], posf[:], start=True,
                             stop=True)
            for dst, shift in ((sinS, 0.0), (cosR, 0.25)):
                tsh = rp.tile([128, 512], F32, tag="tsh")
                nc.vector.tensor_scalar_add(tsh[:], turns[:], shift)
                ti = rp.tile([128, 512], I32, tag="ti")
                nc.vector.tensor_copy(ti[:], tsh[:])
                tif = rp.tile([128, 512], F32, tag="tif")
                nc.vector.tensor_copy(tif[:], ti[:])
                nc.vector.tensor_sub(tsh[:], tsh[:], tif[:])
                nc.scalar.activation(dst[:, ts(ch, 512)], tsh[:], AF.Sin,
                                     scale=TWO_PI)
            # rotate-half sign: rows 0-31 and 64-95 carry -sin
            for q in (0, 2):
                nc.vector.tensor_scalar_mul(
                    sinS[ds(32 * q, 32), ts(ch, 512)],
                    sinS[ds(32 * q, 32), ts(ch, 512)], -1.0)
            if ch == 0:
                # preload the exp table set off the critical path
                dummy = rp.tile([32, 32], BF16, tag="dummy_exp")
                nc.scalar.activation(dummy[:], sinS[ds(0, 32), ds(0, 32)],
                                     AF.Exp)

        # ---- projection helpers ----
        def kq_slice(dst, w_sb, b_b, e, i):
            # one 512-col T-layout projection slice for e-chunk e
            p = prps.tile([128, 512], F32, tag="proj", name="proj_p")
            for dc in range(DC):
                nc.tensor.matmul(p[:], w_sb[:, ds(dc * D + e * 128, 128)],
                                 xT[dc][:, ts(i, 512)],
                                 start=(dc == 0), stop=False)
            nc.tensor.matmul(p[:], b_b[:, ts(e, 128)], ones_row[:],
                             start=False, stop=True)
            nc.vector.tensor_copy(dst[e][:, ts(i, 512)], p[:])

        def rope_slice(dst, e, lo, n, eng=None):
            # in-place rope on dst[e][:, lo:lo+n] using cosR/sinS cols
            sh = shp_pool.tile([128, 1024], BF16, tag="shift", name="sh")
            for q in range(4):
                src_q = (q // 2) * 2 + (1 - q % 2)  # 0<->32, 64<->96
                (eng or nc.sync).dma_start(sh[ds(32 * q, 32), 0:n],
                                           dst[e][ds(32 * src_q, 32),
                                                  ds(lo, n)])
            tmp = shp_pool.tile([128, 1024], BF16, tag="ropetmp",
                                name="ropetmp")
            nc.vector.tensor_mul(tmp[:, 0:n], sh[:, 0:n],
                                 sinS[:, ds(lo, n)])
            nc.vector.tensor_mul(dst[e][:, ds(lo, n)], dst[e][:, ds(lo, n)],
                                 cosR[:, ds(lo, n)])
            nc.vector.tensor_add(dst[e][:, ds(lo, n)], dst[e][:, ds(lo, n)],
                                 tmp[:, 0:n])

        def v_proj_nt(st, nt):
            p = prps.tile([128, 512], F32, tag="proj", name="vproj_p")
            for dc in range(DC):
                nc.tensor.matmul(p[:, 0:384], xT[dc][:, ts(st, 128)],
                                 wv_sb[:, ds(dc * D + nt * 384, 384)],
                                 start=(dc == 0), stop=False)
            nc.tensor.matmul(p[:, 0:384], ones_row[:, 0:128],
                             bv_b[:, ts(nt, 384)], start=False, stop=True)
            dst = Vaug[st][:, 0:H * 65].rearrange("p (h x) -> p h x", x=65)
            nc.vector.tensor_copy(
                dst[:, ds(nt * 6, 6), 0:64],
                p[:, 0:384].rearrange("p (h hd) -> p h hd", hd=64))

        def v_finish(st):
            va = Vaug[st][:, 0:H * 65].rearrange("p (h x) -> p h x", x=65)
            nc.gpsimd.memset(va[:, :, 64:65], 1.0)
            nc.gpsimd.memset(Vaug[st][:, H * 65:], 0.0)

        # ---- prologue: e=0 Q then K (pipelined w/ transposes) then V;
        # rope-shift DMAs ride the idle scalar HWDGE ring ----
        for i in range(2):
            kq_slice(QT, wq_sb, bq_b, 0, i)
            rope_slice(QT, 0, i * 512, 512, eng=nc.scalar)
        for sg in range(4):
            kq_slice(KT, wk_sb, bk_b, 0, sg)
            rope_slice(KT, 0, sg * 512, 512, eng=nc.scalar)
        for st in range(4):
            for nt in range(2):
                v_proj_nt(st, nt)
            v_finish(st)

    def v_all():
        for st in range(4, ST):
            for nt in range(2):
                v_proj_nt(st, nt)
            v_finish(st)

    # wo loads late; its pool reuses the (closed) rope scratch range
    wop = ctx.enter_context(tc.tile_pool(name="wop", bufs=1))
    wo_sb = load_weight(woT, "o", wop)

    # ---- deferred work closures (drained during SDPA) ----
    def make_work(hp):
        items = deque()
        if hp < HP - 1:
            e = hp + 1
            for sg in range(4):
                items.append(lambda e=e, sg=sg: kq_slice(KT, wk_sb, bk_b,
                                                         e, sg))
                items.append(lambda e=e, sg=sg: rope_slice(KT, e, sg * 512,
                                                           512))
            for i in range(2):
                items.append(lambda e=e, i=i: kq_slice(QT, wq_sb, bq_b,
                                                       e, i))
                items.append(lambda e=e, i=i: rope_slice(QT, e, i * 512,
                                                         512))
        return items

    _ocnt = [0]

    def o_tile(st, nt):
        # alternate psum pools so 4 o-groups pipeline in the tail (the
        # score banks are idle by the time these run)
        if _ocnt[0] % 2 == 0:
            p = prps.tile([128, 512], F32, tag="proj", name="o_p")
        else:
            p = scps.tile([128, 512], F32, tag="sc", name="o_p2")
        _ocnt[0] += 1
        for dc in range(DC):
            nc.tensor.matmul(p[:, 0:384], attnT[dc][:, ts(st, 128)],
                             wo_sb[:, ds(dc * D + nt * 384, 384)],
                             start=(dc == 0), stop=(dc == DC - 1))
        o = osb.tile([128, 384], F32, tag="o_out", name="o_out")
        nc.vector.tensor_copy(o[:], p[:, 0:384])
        nc.sync.dma_start(out[ts(st, 128), ts(nt, 384)], o[:])

    # ---- SDPA ----
    pending_norm = [None]
    for hp in range(HP):
        work = make_work(hp)
        steps_total = 32
        step_no = 0
        for qh in range(2):
            pv = [pvps.tile([128, 512], F32, tag=f"pv{i}", name=f"pv{i}")
                  for i in range(2)]
            exs = [None] * ST

            def do_pv(k, pv=pv, exs=exs, hp=hp):
                for i in range(2):
                    h = 2 * hp + i
                    nc.tensor.matmul(pv[i][ds(0, 65), :],
                                     Vaug[k][:, ds(h * 65, 65)],
                                     exs[k][:, ds(i * 512, 512)],
                                     start=(k == 0), stop=(k == ST - 1))

            for skt in range(ST):
                sc = scps.tile([128, 1024], F32, tag="sc", name="sc")
                for i in range(2):
                    nc.tensor.matmul(
                        sc[:, ds(i * 512, 512)],
                        KT[hp][ds(64 * i, 64), ts(skt, 128)],
                        QT[hp][ds(64 * i, 64), ds(qh * 512, 512)],
                        start=True, stop=True, tile_position=(64 * i, 0))
                e = expool.tile([128, 1024], BF16, tag="ex", name="expt")
                nc.scalar.activation(e[:], sc[:], AF.Exp, scale=0.125)
                exs[skt] = e
                if dbg is not None and hp == 0 and qh == 0:
                    nc.sync.dma_start(dbg["ex_d"][ts(skt, 128), :], e[:])
                if skt == 2 and pending_norm[0] is not None:
                    # previous sub-iteration's normalize, emitted here so
                    # its psum reads execute with slack
                    pending_norm[0]()
                    pending_norm[0] = None
                if not (hp == 0 and qh == 0) and skt >= LAG:
                    do_pv(skt - LAG)
                # drain deferred projection work into the PE gap
                step_no += 1
                n_drain = min(len(work), 1)
                for _ in range(n_drain):
                    work.popleft()()
            if hp == 0 and qh == 0:
                # V projections emitted AFTER the qh0 score/exp stream (so
                # scores outrank them on the PE) and BEFORE its PV, which
                # then executes promptly once Vaug lands
                v_all()
                for k in range(ST):
                    do_pv(k)
            else:
                for k in range(ST - LAG, ST):
                    do_pv(k)
            if hp == HP - 1 and qh == 1:
                # o_proj for the qh=0 output rows can start as soon as the
                # last qh=0 norm (emitted at skt==2 above) lands
                for st_ in range(4):
                    for nt_ in range(2):
                        o_tile(st_, nt_)

            # normalize (deferred): evacuate psum, rowsum recip, scale
            def norm_fn(pv=pv, hp=hp, qh=qh):
                for i in range(2):
                    rsrow = nrm.tile([1, 512], F32, tag="rsrow",
                                     name="rsrow")
                    nc.vector.tensor_copy(rsrow[:], pv[i][ds(64, 1), :])
                    pvs = nrm.tile([64, 512], F32, tag="pvs", name="pvs")
                    nc.vector.tensor_copy(pvs[:], pv[i][ds(0, 64), :])
                    c4 = nrm.tile([128, 4], F32, tag="c4", name="c4")
                    nc.sync.dma_start(c4[:], rsrow[:])
                    r4 = nrm.tile([128, 4], F32, tag="r4", name="r4")
                    nc.vector.reciprocal(r4[:], c4[:])
                    recb = nrm.tile([1, 512], F32, tag="recb", name="recb")
                    nc.sync.dma_start(recb[:], r4[:])
                    rbs = nrm.tile([64, 512], F32, tag="rbs", name="rbs")
                    nc.gpsimd.partition_broadcast(rbs[:], recb[:])
                    nc.vector.tensor_mul(
                        attnT[hp][ds(64 * i, 64), ds(qh * 512, 512)],
                        pvs[:], rbs[:])
                    if dbg is not None and hp == 0:
                        blk = (qh * 2 + i)
                        nc.sync.dma_start(
                            dbg["pv_d"][ds(blk * 65, 64), :], pvs[:])
                        nc.sync.dma_start(
                            dbg["pv_d"][ds(blk * 65 + 64, 1), :], rsrow[:])
                        nc.sync.dma_start(
                            dbg["rb_d"][ds(blk * 64, 64), :], rbs[:])

            pending_norm[0] = norm_fn
    pending_norm[0]()

    # ---- o_proj remainder (qh=1 output rows) ----
    for st in range(4, SQ // 128):
        for nt in range(2):
            o_tile(st, nt)

    if dbg is not None:
        for dc in range(DC):
            nc.sync.dma_start(dbg["xt_d"][ts(dc, 128), :], xT[dc][:])
            nc.sync.dma_start(dbg["kt_d"][ts(dc, 128), :], KT[dc][:])
            nc.sync.dma_start(dbg["qt_d"][ts(dc, 128), :], QT[dc][:])
            nc.sync.dma_start(dbg["at_d"][ts(dc, 128), :], attnT[dc][:])
        for st in range(ST):
            nc.sync.dma_start(dbg["va_d"][ts(st, 128), :], Vaug[st][:])


_NC_CACHE = None


def _get_nc():
    global _NC_CACHE
    if _NC_CACHE is None:
        _NC_CACHE = build_nc()
    return _NC_CACHE


def kernel(hidden_states, position_ids, wq, bq, wk, bk, wv, bv, wo,
           _trace=False):
    import ml_dtypes
    bf16 = ml_dtypes.bfloat16
    hidden_states = np.asarray(hidden_states, dtype=np.float32).astype(bf16)
    position_ids = np.asarray(position_ids, dtype=np.int32)
    wqT = np.ascontiguousarray(np.asarray(wq, np.float32).T.astype(bf16))
    wkT = np.ascontiguousarray(np.asarray(wk, np.float32).T.astype(bf16))
    wvT = np.ascontiguousarray(np.asarray(wv, np.float32).T.astype(bf16))
    woT = np.ascontiguousarray(np.asarray(wo, np.float32).T.astype(bf16))
    bq_r = np.ascontiguousarray(np.asarray(bq, np.float32).reshape(1, D))
    bk_r = np.ascontiguousarray(np.asarray(bk, np.float32).reshape(1, D))
    bv_r = np.ascontiguousarray(np.asarray(bv, np.float32).reshape(1, D))

    nc = _get_nc()
    in_maps = []
    for core in range(N_CORES):
        b, half = core // 2, core % 2
        if half == 0:
            hsp = hidden_states[b]
            posp = position_ids[b]
        else:
            hsp = np.concatenate([hidden_states[b, SQ:],
                                  hidden_states[b, :SQ]], axis=0)
            posp = np.concatenate([position_ids[b, SQ:],
                                   position_ids[b, :SQ]])
        in_maps.append({
            "hs": np.ascontiguousarray(hsp),
            "pos": np.ascontiguousarray(posp.reshape(1, S)),
            "wqT": wqT, "wkT": wkT, "wvT": wvT, "woT": woT,
            "bq": bq_r, "bk": bk_r, "bv": bv_r,
        })
    res = run_bass_kernel_spmd(nc, in_maps, list(range(N_CORES)),
                               trace=_trace)
    if DEBUG:
        kernel._last_dbg_results = res
    outp = np.empty((B, S, D), np.float32)
    for core in range(N_CORES):
        b, half = core // 2, core % 2
        outp[b, half * SQ:(half + 1) * SQ] = res.results[core]["out"]
    if _trace:
        kernel._last_exec_time_ns = res.exec_time_ns
        kernel._last_results = res
    return outp
